# revision 4
# baseline (speedup 1.0000x reference)
"""Trainium2 Bass kernel for nn_DSSMReverse (DSSM embed/conv/VQ/Gram model).

Strategy: data-parallel over batch across 8 NeuronCores (128 images each).
 - Embedding+conv0 are composed on host into a one-hot conv; one-hot planes
   (with 3 row-shifted groups for tap packing) are prepared on host.
 - All 3x3 convs run as shifted-window matmuls with row-group packing
   (K=42/96/128+64) to keep the 128x128 PE array well fed.
 - phi conv1 outputs [128c x 400px] spill to DRAM, then the 51200->512
   linears run as 400 pixel-matmuls (M=128 images, N=512 outs) with the
   100MB weight matrices streamed from HBM.
 - Phase C (feat-major [4x128, B]): VQ codebook argmax+gather via matmuls +
   PE transposes, fc stacks, L2 normalize via ones-matmul partition
   reduction, AllGather of sp_out across cores, per-core [128, 1024] block
   of the final Gram matrix (exp(scale) folded into the s-side scale).
"""

import os
import numpy as np

import concourse.bacc as bacc
import concourse.bass as bass
import concourse.mybir as mybir
import concourse.tile as tile
from concourse.masks import make_identity

N_CORES = 8
B_FULL = 1024
H = W = 20
PW = 22            # padded plane width
NPIX = 400
NPAD = 484         # 22*22
NVOC = 14
EPS = 1e-4
F32 = mybir.dt.float32
AF = mybir.ActivationFunctionType
ALU = mybir.AluOpType

G_PIX = 100        # pixels per x-tile in phase B
W_PIX = 16         # pixels per streamed weight tile in phase B


# --------------------------------------------------------------------------
# device program
# --------------------------------------------------------------------------

def build_program(BL):
    nc = bacc.Bacc("TRN2", target_bir_lowering=False, debug=False,
                   num_devices=N_CORES)

    def inp(name, shape):
        return nc.dram_tensor(name, shape, F32, kind="ExternalInput").ap()

    oh_s = inp("oh_s", [BL, 42, NPAD])
    oh_sp = inp("oh_sp", [BL, 42, NPAD])
    wE0 = inp("wE0", [42, 3, 32])
    wE1 = inp("wE1", [96, 3, 64])
    wA0 = inp("wA0", [128, 9, 64])
    wB0 = inp("wB0", [128, 9, 64])
    wA1 = inp("wA1", [128, 9, 128])
    wB1 = inp("wB1", [128, 9, 128])
    b0 = inp("b0", [32, 1])
    b1 = inp("b1", [64, 1])
    bc0 = inp("bc0", [64, 3])
    bc1 = inp("bc1", [128, 3])
    wl = inp("wl", [3, NPIX, 128, 512])
    blpk = inp("blpk", [128, 3, 4])
    fcT = {n: inp(n + "T", [128, 4, 512]) for n in ("fc0", "fc1", "p3f0", "p3f1")}
    fcB = {n: inp(n + "b", [128, 4]) for n in ("fc0", "fc1", "p3f0", "p3f1")}
    zT = inp("zT", [128, 4, 64])
    zrow = inp("zrow", [64, 512])
    zsq = inp("zsq", [64, 1])
    esc = inp("esc", [1, 1])

    out_d = nc.dram_tensor("out", [BL, N_CORES * BL], F32,
                           kind="ExternalOutput").ap()

    cpc = min(N_CORES, 512 // BL)          # gram col-cores per matmul chunk
    n_chunks = (N_CORES + cpc - 1) // cpc

    with tile.TileContext(nc) as tc:
        with (
            tc.tile_pool(name="const", bufs=1) as cst,
            tc.tile_pool(name="inter", bufs=1) as inter,
            tc.tile_pool(name="dram", bufs=1, space="DRAM") as dram,
        ):
            # ---- persistent weights -> SBUF
            def load(ap, shape, tag):
                t = cst.tile(shape, F32, tag=tag, name=tag)
                nc.sync.dma_start(out=t[:], in_=ap[:])
                return t

            wE0_s = load(wE0, [42, 3, 32], "wE0")
            wE1_s = load(wE1, [96, 3, 64], "wE1")
            wA0_s = load(wA0, [128, 9, 64], "wA0")
            wB0_s = load(wB0, [128, 9, 64], "wB0")
            wA1_s = load(wA1, [128, 9, 128], "wA1")
            wB1_s = load(wB1, [128, 9, 128], "wB1")
            b0_s = load(b0, [32, 1], "b0")
            b1_s = load(b1, [64, 1], "b1")
            bc0_s = load(bc0, [64, 3], "bc0")
            bc1_s = load(bc1, [128, 3], "bc1")
            bl_s = load(blpk, [128, 3, 4], "blpk")
            fcT_s = {n: load(fcT[n], [128, 4, 512], n + "T") for n in fcT}
            fcB_s = {n: load(fcB[n], [128, 4], n + "b") for n in fcB}
            zT_s = load(zT, [128, 4, 64], "zT")
            zrow_s = load(zrow, [64, 512], "zrow")
            zsq_s = load(zsq, [64, 1], "zsq")
            esc_s = load(esc, [1, 1], "esc")

            ident = cst.tile([128, 128], F32, tag="ident")
            make_identity(nc, ident[:])
            ones_col = cst.tile([128, 1], F32, tag="ones_col")
            nc.vector.memset(ones_col[:], 1.0)
            ones_row = cst.tile([1, 128], F32, tag="ones_row")
            nc.vector.memset(ones_row[:], 1.0)

            # DRAM spill for phi conv1 outputs, layout [c, img, px]
            xsp = [dram.tile([128, BL, NPIX], F32, tag=f"xsp{i}",
                             name=f"xsp{i}") for i in range(3)]
            # collective buffers
            spT_d = dram.tile([128, 4, BL], F32, tag="spT")
            spall = dram.tile([N_CORES, 128, 4, BL], F32, tag="spall",
                              addr_space="Shared")

            # ============================================================
            # phase A: conv stacks, image by image
            # ============================================================
            with (
                tc.tile_pool(name="pa", bufs=2) as pa,
                tc.tile_pool(name="pa3", bufs=3) as pa3,
                tc.tile_pool(name="paps", bufs=2, space="PSUM") as paps,
            ):
                for img in range(BL):
                    embbig = {}
                    for m, oh_ap in (("s", oh_s), ("sp", oh_sp)):
                        oh = pa.tile([42, NPAD], F32, tag="oh")
                        nc.sync.dma_start(out=oh[:], in_=oh_ap[img])
                        ohv = oh[:].rearrange("p (y x) -> p y x", y=PW, x=PW)

                        # conv0 (one-hot composed, K=42, 3 taps) -> [32,400]
                        pe0 = paps.tile([32, NPIX], F32, tag="pe0")
                        for kx in range(3):
                            nc.tensor.matmul(
                                pe0[:], wE0_s[:, kx, :],
                                ohv[:, 0:20, kx:kx + 20],
                                start=(kx == 0), stop=(kx == 2))

                        # evacuate 3 row-shifted relu copies -> e0big [96,484]
                        e0big = pa.tile([96, NPAD], F32, tag="e0big")
                        nc.gpsimd.memset(e0big[:], 0.0)
                        ev = e0big[:].rearrange("p (y x) -> p y x", y=PW, x=PW)
                        p0v = pe0[:].rearrange("p (y x) -> p y x", y=20, x=20)
                        nc.scalar.activation(ev[0:32, 1:21, 1:21], pe0[:],
                                             AF.Relu, bias=b0_s[:])
                        nc.vector.tensor_scalar(ev[32:64, 0:20, 1:21], pe0[:],
                                                b0_s[:], 0.0, ALU.add, ALU.max)
                        nc.scalar.activation(ev[64:96, 0:19, 1:21],
                                             p0v[:, 1:20, :],
                                             AF.Relu, bias=b0_s[:])

                        # conv1 (K=96, 3 taps) -> [64, 400]
                        pe1 = paps.tile([64, NPIX], F32, tag="pe1")
                        e0v = e0big[:].rearrange("p (y x) -> p y x", y=PW, x=PW)
                        for kx in range(3):
                            nc.tensor.matmul(
                                pe1[:], wE1_s[:, kx, :],
                                e0v[:, 0:20, kx:kx + 20],
                                start=(kx == 0), stop=(kx == 2))

                        # evacuate 2 row-shifted relu copies -> embbig [128,484]
                        eb = pa.tile([128, NPAD], F32, tag=f"emb_{m}")
                        nc.gpsimd.memset(eb[:], 0.0)
                        ebv = eb[:].rearrange("p (y x) -> p y x", y=PW, x=PW)
                        nc.scalar.activation(ebv[0:64, 1:21, 1:21], pe1[:],
                                             AF.Relu, bias=b1_s[:])
                        nc.vector.tensor_scalar(ebv[64:128, 0:20, 1:21], pe1[:],
                                                b1_s[:], 0.0, ALU.add, ALU.max)
                        embbig[m] = eb

                    diff = pa.tile([128, NPAD], F32, tag="diff")
                    nc.vector.tensor_tensor(diff[:], embbig["sp"][:],
                                            embbig["s"][:], ALU.subtract)

                    for phi, src in enumerate((embbig["s"], diff,
                                               embbig["sp"])):
                        sv = src[:].rearrange("p (y x) -> p y x", y=PW, x=PW)
                        # conv0: 3 pair-taps (K=128) + 3 leftover (K=64)
                        pc0 = paps.tile([64, NPIX], F32, tag="pc0")
                        for kx in range(3):
                            nc.tensor.matmul(
                                pc0[:], wA0_s[:, phi * 3 + kx, :],
                                sv[:, 0:20, kx:kx + 20],
                                start=(kx == 0), stop=False)
                        for kx in range(3):
                            nc.tensor.matmul(
                                pc0[:], wB0_s[64:128, phi * 3 + kx, :],
                                sv[64:128, 1:21, kx:kx + 20],
                                start=False, stop=(kx == 2))

                        c0big = pa.tile([128, NPAD], F32, tag="c0big")
                        nc.gpsimd.memset(c0big[:], 0.0)
                        cv = c0big[:].rearrange("p (y x) -> p y x", y=PW, x=PW)
                        nc.scalar.activation(cv[0:64, 1:21, 1:21], pc0[:],
                                             AF.Relu, bias=bc0_s[:, phi:phi + 1])
                        nc.vector.tensor_scalar(cv[64:128, 0:20, 1:21], pc0[:],
                                                bc0_s[:, phi:phi + 1], 0.0,
                                                ALU.add, ALU.max)

                        # conv1: 3 pair-taps (K=128) + 3 leftover (K=64)
                        pc1 = paps.tile([128, NPIX], F32, tag="pc1")
                        c0v = c0big[:].rearrange("p (y x) -> p y x", y=PW, x=PW)
                        for kx in range(3):
                            nc.tensor.matmul(
                                pc1[:], wA1_s[:, phi * 3 + kx, :],
                                c0v[:, 0:20, kx:kx + 20],
                                start=(kx == 0), stop=False)
                        for kx in range(3):
                            nc.tensor.matmul(
                                pc1[:], wB1_s[64:128, phi * 3 + kx, :],
                                c0v[64:128, 1:21, kx:kx + 20],
                                start=False, stop=(kx == 2))

                        c1sb = pa3.tile([128, NPIX], F32, tag="c1sb")
                        nc.scalar.activation(c1sb[:], pc1[:], AF.Relu,
                                             bias=bc1_s[:, phi:phi + 1])
                        nc.sync.dma_start(out=xsp[phi][:, img, :],
                                          in_=c1sb[:])

            # ============================================================
            # phase B: 51200->512 linears (+ transpose to feat-major)
            # phase C interleaved: quantize after phi=1, fc stacks after
            # ============================================================
            xT = {}      # feat-major [128, 4, BL] phi outputs (bias added)

            with (
                tc.tile_pool(name="pb", bufs=2) as pb,
                tc.tile_pool(name="pbps", bufs=2, space="PSUM") as pbps,
                tc.tile_pool(name="pc", bufs=1) as pc,
                tc.tile_pool(name="pcps", bufs=4, space="PSUM") as pcps,
            ):
                def linear_phi(phi):
                    acc = pbps.tile([BL, 512], F32, tag="acc")
                    for g in range(NPIX // G_PIX):
                        xg = pb.tile([128, BL, G_PIX], F32, tag="xg")
                        nc.sync.dma_start(
                            out=xg[:],
                            in_=xsp[phi][:, :, g * G_PIX:(g + 1) * G_PIX])
                        for wchunk in range(G_PIX // W_PIX + (1 if G_PIX % W_PIX else 0)):
                            p_lo = wchunk * W_PIX
                            p_hi = min(G_PIX, p_lo + W_PIX)
                            if p_lo >= p_hi:
                                continue
                            wt = pb.tile([128, W_PIX, 512], F32, tag="wt")
                            nc.sync.dma_start(
                                out=wt[:, 0:p_hi - p_lo, :],
                                in_=wl[phi, g * G_PIX + p_lo:g * G_PIX + p_hi]
                                .rearrange("n c o -> c n o"))
                            for j in range(p_hi - p_lo):
                                gp = g * G_PIX + p_lo + j
                                nc.tensor.matmul(
                                    acc[:], xg[:, :, p_lo + j], wt[:, j, :],
                                    start=(gp == 0), stop=(gp == NPIX - 1))
                    # PSUM [BL, 512] -> SBUF, transpose to [128, 4, BL] + bias
                    asb = pc.tile([BL, 512], F32, tag=f"asb{phi}")
                    nc.scalar.copy(asb[:], acc[:])
                    t = inter.tile([128, 4, BL], F32, tag=f"xT{phi}")
                    for k in range(4):
                        pt = pcps.tile([128, 512], F32, tag="ps")
                        nc.tensor.transpose(pt[:, 0:BL],
                                            asb[:, k * 128:(k + 1) * 128],
                                            ident[0:BL, 0:BL])
                        nc.vector.tensor_scalar(t[:, k, :], pt[:, 0:BL],
                                                bl_s[:, phi, k:k + 1], None, ALU.add)
                    xT[phi] = t

                def fc_layer(h_in, wname, relu, tag):
                    h_out = pc.tile([128, 4, BL], F32, tag=tag)
                    for j in range(4):
                        ps = pcps.tile([128, 512], F32, tag="ps")
                        for k in range(4):
                            nc.tensor.matmul(
                                ps[:, 0:BL],
                                fcT_s[wname][:, k, j * 128:(j + 1) * 128],
                                h_in[:, k, :],
                                start=(k == 0), stop=(k == 3))
                        if relu:
                            nc.vector.tensor_scalar(
                                h_out[:, j, :], ps[:, 0:BL],
                                fcB_s[wname][:, j:j + 1], 0.0, ALU.add, ALU.max)
                        else:
                            nc.vector.tensor_scalar(
                                h_out[:, j, :], ps[:, 0:BL],
                                fcB_s[wname][:, j:j + 1], None, ALU.add)
                    return h_out

                def normalize(h_in, with_escale, tag):
                    # returns h_in * 1/(||h||+eps) [* exp(scale)]
                    sq = pc.tile([128, 4, BL], F32, tag=tag + "_sq")
                    nc.vector.tensor_tensor(sq[:], h_in[:], h_in[:], ALU.mult)
                    pn = pcps.tile([128, 512], F32, tag="ps")
                    for k in range(4):
                        nc.tensor.matmul(pn[0:1, 0:BL], ones_col[:],
                                         sq[:, k, :],
                                         start=(k == 0), stop=(k == 3))
                    tn = pc.tile([1, BL], F32, tag=tag + "_tn")
                    nc.scalar.activation(tn[:], pn[0:1, 0:BL], AF.Sqrt)
                    nc.vector.tensor_scalar_add(tn[:], tn[:], EPS)
                    rn = pc.tile([1, BL], F32, tag=tag + "_rn")
                    nc.vector.reciprocal(rn[:], tn[:])
                    if with_escale:
                        nc.vector.tensor_scalar_mul(rn[:], rn[:], esc_s[:])
                    pbx = pcps.tile([128, 512], F32, tag="ps")
                    nc.tensor.matmul(pbx[:, 0:BL], ones_row[:], rn[:],
                                     start=True, stop=True)
                    h_out = pc.tile([128, 4, BL], F32, tag=tag)
                    for k in range(4):
                        nc.vector.tensor_tensor(h_out[:, k, :], h_in[:, k, :],
                                                pbx[:, 0:BL], ALU.mult)
                    return h_out

                # ---- diff first (feeds the longest chain: quantize)
                linear_phi(1)

                # quantize: scoreT[j,b] = zsq_j - 2 * (z @ diff)[j,b]
                pG = pcps.tile([128, 512], F32, tag="ps")
                for k in range(4):
                    nc.tensor.matmul(pG[0:64, 0:BL], zT_s[:, k, :],
                                     xT[1][:, k, :],
                                     start=(k == 0), stop=(k == 3))
                scT = pc.tile([64, BL], F32, tag="scT")
                nc.scalar.activation(scT[:], pG[0:64, 0:BL], AF.Identity,
                                     bias=zsq_s[:], scale=-2.0)
                pSc = pcps.tile([128, 512], F32, tag="ps")
                nc.tensor.transpose(pSc[0:BL, 0:64], scT[:], ident[0:64, 0:64])
                scB = pc.tile([BL, 64], F32, tag="scB")
                nc.vector.tensor_copy(scB[:], pSc[0:BL, 0:64])
                mx = pc.tile([BL, 1], F32, tag="mx")
                nc.vector.tensor_reduce(mx[:], scB[:], mybir.AxisListType.X,
                                        ALU.max)
                ohB = pc.tile([BL, 64], F32, tag="ohB")
                nc.vector.tensor_scalar(ohB[:], scB[:], mx[:], None, ALU.is_ge)
                pOh = pcps.tile([128, 512], F32, tag="ps")
                nc.tensor.transpose(pOh[0:64, 0:BL], ohB[:],
                                    ident[0:BL, 0:BL])
                ohT = pc.tile([64, BL], F32, tag="ohT")
                nc.vector.tensor_copy(ohT[:], pOh[0:64, 0:BL])

                # ---- s path linear
                linear_phi(0)

                # z_matrix gather + add s_int
                h0 = pc.tile([128, 4, BL], F32, tag="h0")
                for k in range(4):
                    pz = pcps.tile([128, 512], F32, tag="ps")
                    nc.tensor.matmul(pz[:, 0:BL],
                                     zrow_s[:, k * 128:(k + 1) * 128],
                                     ohT[:], start=True, stop=True)
                    nc.vector.scalar_tensor_tensor(
                        h0[:, k, :], pz[:, 0:BL], 0.0, xT[0][:, k, :],
                        ALU.bypass, ALU.add)

                h1 = fc_layer(h0, "fc0", True, "h1")
                h2 = fc_layer(h1, "fc1", False, "h2")
                s_outT = normalize(h2, True, "soT")

                # ---- sp path linear + fc stack
                linear_phi(2)
                g1 = fc_layer(xT[2], "p3f0", True, "g1")
                g2 = fc_layer(g1, "p3f1", False, "g2")
                sp_outT = normalize(g2, False, "spoT")

                # ---- allgather sp_out across cores
                nc.sync.dma_start(out=spT_d[:], in_=sp_outT[:])
                nc.gpsimd.collective_compute(
                    "AllGather", ALU.bypass,
                    replica_groups=[list(range(N_CORES))],
                    ins=[spT_d[:]], outs=[spall[:]])

                # ---- gram block: out[my_b, all_b]
                outsb = pc.tile([BL, N_CORES * BL], F32, tag="outsb")
                spv = spall[:].rearrange("c p k b -> p k c b")
                for h in range(n_chunks):
                    ncol = cpc * BL
                    pi = pcps.tile([128, 512], F32, tag="ps")
                    for k in range(4):
                        sps = pb.tile([128, cpc, BL], F32, tag="sps")
                        nc.sync.dma_start(
                            out=sps[:],
                            in_=spv[:, k, h * cpc:(h + 1) * cpc, :])
                        spsf = sps[:].rearrange("p c b -> p (c b)")
                        nc.tensor.matmul(pi[0:BL, 0:ncol], s_outT[:, k, :],
                                         spsf, start=(k == 0), stop=(k == 3))
                    nc.scalar.copy(outsb[:, h * ncol:(h + 1) * ncol],
                                   pi[0:BL, 0:ncol])
                nc.sync.dma_start(out=out_d[:], in_=outsb[:])

    nc.finalize()
    return nc


# --------------------------------------------------------------------------
# host-side input preparation
# --------------------------------------------------------------------------

def _onehot42(idx):
    """[n,20,20] int -> [n,42,484] f32 one-hot planes, 3 row-shifted groups."""
    n = idx.shape[0]
    pad = np.full((n, PW, PW), -1, np.int32)
    pad[:, 1:21, 1:21] = idx
    flat = pad.reshape(n, NPAD)
    out = np.zeros((n, 42, NPAD), np.float32)
    cls = np.arange(NVOC, dtype=np.int32)
    for ky in range(3):
        L = NPAD - PW * ky
        sh = flat[:, PW * ky:]                       # [n, L]
        out[:, ky * NVOC:(ky + 1) * NVOC, :L] = (
            sh[:, None, :] == cls[None, :, None]).astype(np.float32)
    return out


def prep_shared(inputs):
    f = np.float32
    t = {}
    emb = np.asarray(inputs["emb_table"], f)
    norms = np.linalg.norm(emb, axis=1, keepdims=True)
    tbl = emb * np.where(norms > 1.0, f(1.0) / (norms + f(1e-7)), f(1.0))

    # composed one-hot conv0 weights: M[c,o,ky,kx] = sum_e ec0_w[o,e,ky,kx]*tbl[c,e]
    M = np.einsum("oeyx,ce->coyx", np.asarray(inputs["ec0_w"], np.float64),
                  tbl.astype(np.float64)).astype(f)       # [14,32,3,3]
    wE0 = np.zeros((42, 3, 32), f)
    for ky in range(3):
        for kx in range(3):
            wE0[ky * 14:(ky + 1) * 14, kx, :] = M[:, :, ky, kx]
    t["wE0"] = wE0

    e1 = np.asarray(inputs["ec1_w"], f)                   # [64,32,3,3]
    wE1 = np.zeros((96, 3, 64), f)
    for ky in range(3):
        for kx in range(3):
            wE1[ky * 32:(ky + 1) * 32, kx, :] = e1[:, :, ky, kx].T
    t["wE1"] = wE1

    wA0 = np.zeros((128, 9, 64), f)
    wB0 = np.zeros((128, 9, 64), f)
    wA1 = np.zeros((128, 9, 128), f)
    wB1 = np.zeros((128, 9, 128), f)
    for phi, p in enumerate(("p1", "p2", "p3")):
        c0 = np.asarray(inputs[p + "c0_w"], f)            # [64,64,3,3]
        c1 = np.asarray(inputs[p + "c1_w"], f)            # [128,64,3,3]
        for kx in range(3):
            wA0[0:64, phi * 3 + kx, :] = c0[:, :, 0, kx].T
            wA0[64:128, phi * 3 + kx, :] = c0[:, :, 1, kx].T
            wB0[64:128, phi * 3 + kx, :] = c0[:, :, 2, kx].T
            wA1[0:64, phi * 3 + kx, :] = c1[:, :, 0, kx].T
            wA1[64:128, phi * 3 + kx, :] = c1[:, :, 1, kx].T
            wB1[64:128, phi * 3 + kx, :] = c1[:, :, 2, kx].T
    t["wA0"], t["wB0"], t["wA1"], t["wB1"] = wA0, wB0, wA1, wB1

    t["b0"] = np.asarray(inputs["ec0_b"], f).reshape(32, 1)
    t["b1"] = np.asarray(inputs["ec1_b"], f).reshape(64, 1)
    t["bc0"] = np.stack([np.asarray(inputs[p + "c0_b"], f)
                         for p in ("p1", "p2", "p3")], axis=1)   # [64,3]
    t["bc1"] = np.stack([np.asarray(inputs[p + "c1_b"], f)
                         for p in ("p1", "p2", "p3")], axis=1)   # [128,3]

    # big linear weights: wl[phi, p, c, o] = W[o, c*400+p]
    t["wl"] = np.stack([
        np.ascontiguousarray(
            np.asarray(inputs[p + "l_w"], f).reshape(512, 128, NPIX)
            .transpose(2, 1, 0))
        for p in ("p1", "p2", "p3")])                      # [3,400,128,512]
    t["blpk"] = np.stack([np.asarray(inputs[p + "l_b"], f).reshape(4, 128).T
                          for p in ("p1", "p2", "p3")], axis=1)  # [128,3,4]

    for n in ("fc0", "fc1", "p3f0", "p3f1"):
        w = np.asarray(inputs[n + "_w"], f)               # [512,512] out,in
        t[n + "T"] = np.ascontiguousarray(
            w.T.reshape(4, 128, 512).transpose(1, 0, 2))  # [128,4,512]
        t[n + "b"] = np.asarray(inputs[n + "_b"], f).reshape(4, 128).T

    z = np.asarray(inputs["z_vec"], f)                    # [64,512]
    t["zT"] = np.ascontiguousarray(
        z.T.reshape(4, 128, 64).transpose(1, 0, 2))       # [128,4,64]
    t["zrow"] = z
    t["zsq"] = (z * z).sum(axis=1, keepdims=True).astype(f)
    t["esc"] = np.exp(np.asarray(inputs["scale"], f)).reshape(1, 1)
    return t


def make_in_maps(inputs, BL):
    shared = prep_shared(inputs)
    s = np.asarray(inputs["s"])
    sp = np.asarray(inputs["s_prime"])
    maps = []
    for c in range(N_CORES):
        m = dict(shared)
        m["oh_s"] = _onehot42(s[c * BL:(c + 1) * BL])
        m["oh_sp"] = _onehot42(sp[c * BL:(c + 1) * BL])
        maps.append(m)
    return maps


# --------------------------------------------------------------------------
# cached runner (jit once, reuse across calls)
# --------------------------------------------------------------------------

class Runner:
    def __init__(self, BL):
        import jax
        from jax.sharding import Mesh, PartitionSpec, NamedSharding
        from jax.experimental.shard_map import shard_map
        from concourse import bass2jax
        self.BL = BL
        self.nc = build_program(BL)
        nc = self.nc
        bass2jax.install_neuronx_cc_hook()

        partition_name = (nc.partition_id_tensor.name
                          if nc.partition_id_tensor else None)
        in_names, out_names, out_avals, zero_outs = [], [], [], []
        for alloc in nc.m.functions[0].allocations:
            if not isinstance(alloc, mybir.MemoryLocationSet):
                continue
            name = alloc.memorylocations[0].name
            if alloc.kind == "ExternalInput":
                if name != partition_name:
                    in_names.append(name)
            elif alloc.kind == "ExternalOutput":
                shape = tuple(alloc.tensor_shape)
                dtype = mybir.dt.np(alloc.dtype)
                out_names.append(name)
                out_avals.append(jax.core.ShapedArray(shape, dtype))
                zero_outs.append(np.zeros(shape, dtype))
        self.in_names, self.out_names = list(in_names), out_names
        self.out_avals, self.zero_outs = out_avals, zero_outs
        n_params, n_outs = len(in_names), len(out_avals)
        all_in_names = in_names + out_names
        if partition_name is not None:
            all_in_names = all_in_names + [partition_name]

        def _body(*args):
            operands = list(args)
            if partition_name is not None:
                operands.append(bass2jax.partition_id_tensor())
            return tuple(bass2jax._bass_exec_p.bind(
                *operands,
                out_avals=tuple(out_avals),
                in_names=tuple(all_in_names),
                out_names=tuple(out_names),
                lowering_input_output_aliases=(),
                sim_require_finite=True,
                sim_require_nnan=True,
                nc=nc,
            ))

        devices = jax.devices()[:N_CORES]
        self.mesh = Mesh(np.asarray(devices), ("core",))
        in_specs = (PartitionSpec("core"),) * (n_params + n_outs)
        out_specs = (PartitionSpec("core"),) * n_outs
        self.sharding = NamedSharding(self.mesh, PartitionSpec("core"))
        self.jitted = jax.jit(
            shard_map(_body, mesh=self.mesh, in_specs=in_specs,
                      out_specs=out_specs, check_rep=False),
            donate_argnums=tuple(range(n_params, n_params + n_outs)),
            keep_unused=True)
        self._staged = None
        self._jax = jax

    def stage(self, in_maps):
        """device_put the concatenated inputs once."""
        jax = self._jax
        concat = [np.concatenate([np.asarray(m[n]) for m in in_maps], axis=0)
                  for n in self.in_names]
        self._staged = [jax.device_put(a, self.sharding) for a in concat]
        jax.block_until_ready(self._staged)

    def run(self):
        jax = self._jax
        zo = [jax.device_put(
            np.zeros((N_CORES * z.shape[0], *z.shape[1:]), z.dtype),
            self.sharding) for z in self.zero_outs]
        jax.block_until_ready(zo)
        outs = self.jitted(*self._staged, *zo)
        jax.block_until_ready(outs)
        return outs

    def output(self, outs):
        o = np.asarray(outs[self.out_names.index("out")])
        return o  # [N_CORES*BL, N_CORES*BL]


_RUNNER_CACHE = {}


def _get_runner(BL):
    if BL not in _RUNNER_CACHE:
        _RUNNER_CACHE[BL] = Runner(BL)
    return _RUNNER_CACHE[BL]


def kernel(**inputs):
    assert int(np.asarray(inputs["downscale_factor"])) == 1
    BL = np.asarray(inputs["s"]).shape[0] // N_CORES
    r = _get_runner(BL)
    r.stage(make_in_maps(inputs, BL))
    outs = r.run()
    return r.output(outs).astype(np.float32)
